# revision 40
# baseline (speedup 1.0000x reference)
"""Trainium2 Bass kernel for the scatter_memory delta-rule module (v3).

Computation (per batch b, head h):
  Y = X @ [W_mk|W_mv|W_mb].T            (X = mem_tokens[b], [S, D])
  k_raw, new_mv, mb_raw = per-head 64-col slices of Y
  xx  = [relu(k), relu(-k)]             ([S, 128])
  mk_j = xx * roll_j(xx), j=1..3        (mk = [S, 384], all >= 0)
  ss  = ||mk||^2, alpha = rsqrt(ss)
  num = mk @ W_mem, zmk = mk @ z        (retrieval)
  prev = num / zmk   (the reference's 1e-5 eps term is <= 1e-5 relative
                      since zmk = L1(mk) >= L2(mk) = r; dropped)
  mvg = (new_mv - prev) * sigmoid(mb_raw) * alpha
  dW  = mk.T @ mvg ;  out = W_mem + dW

v3: software-pipelined by one tile. Iteration i runs tile i's
projections/relus/phi on tensor/scalar/gpsimd while tile i-1's PE
transposes, PSUM->SBUF copies, retrieval and vector tail execute, and
tile i-2's state has fully drained. PSUM: 4 dW accumulator banks, 1
K/B bank, 3 ring banks cycling [T0 T1 T2 R0 R1 V] so every slot-reuse
dependency is at least one iteration stale. psV is drained early by
vg = psV*g (sigmoid gate available same iteration); alpha lands on the
v-side one iteration later as mvg = vg*alpha - (num*beta*alpha)*g.

Sharding: 8 cores = (4 batches) x (2 half-head groups of 8 heads).
Host prep: X transposed to [D, S] bf16; weight slices pre-transposed;
W_mem/z packed to a [H, 3, 128, 65] rhs. Device returns dW.T [H, 64, 384]
fp32; host transposes and adds W_mem in fp32.
"""

import numpy as np
import ml_dtypes
from contextlib import ExitStack

def _split_excess_waits(nc, max_waits=1, drain_waits=1):
    """The walrus build here encodes only ONE sync wait per instruction
    (updates are separate). Move excess waits onto prepended same-engine
    drains, one wait each."""
    from concourse import mybir

    ctr = [0]
    for f in nc.m.functions:
        for bb in f.blocks:
            il = list(bb.instructions)
            out = []
            changed = False
            for inst in il:
                si = getattr(inst, "sync_info", None)
                waits = list(si.on_wait) if si and si.on_wait else []
                ups = list(si.on_update) if si and si.on_update else []
                if len(waits) > max_waits:
                    keep = waits[:max_waits]
                    rest = waits[max_waits:]
                    for i in range(0, len(rest), drain_waits):
                        chunk = rest[i:i + drain_waits]
                        ctr[0] += 1
                        d = mybir.InstDrain(
                            name=f"waitsplit{ctr[0]}",
                            ins=[],
                            outs=[],
                            bass_is_fusable=False,
                        )
                        d.engine = inst.engine
                        d.sync_info = mybir.SyncInfo(on_wait=chunk, on_update=[])
                        out.append(d)
                    inst.sync_info = mybir.SyncInfo(on_wait=keep, on_update=ups)
                    changed = True
                out.append(inst)
            if changed:
                bb.instructions = out
    return ctr[0]

B, S, D = 4, 4096, 1024
HPC = 8            # heads per core
NCORES = 8
DK = 64            # dk per head
DKEY = 384         # 2*nu*dk
DV = 64
ST = 128           # tokens per tile
NST = S // ST      # 32
NJ = 3


def _body(ctx, tc, out_dwt, xt, wt, rhs, idin):
    import concourse.bass as bass
    from concourse import mybir

    nc = tc.nc
    bf16 = mybir.dt.bfloat16
    f32 = mybir.dt.float32
    i32 = mybir.dt.int32
    Alu = mybir.AluOpType
    Act = mybir.ActivationFunctionType

    singles = ctx.enter_context(tc.tile_pool(name="singles", bufs=1))
    xpool = ctx.enter_context(tc.tile_pool(name="xpool", bufs=4))
    work = ctx.enter_context(tc.tile_pool(name="work", bufs=4))
    tiny = ctx.enter_context(tc.tile_pool(name="tiny", bufs=4))
    kbt = ctx.enter_context(tc.tile_pool(name="kbt", bufs=1, space="PSUM"))
    ring = ctx.enter_context(tc.tile_pool(name="ring", bufs=3, space="PSUM"))
    dpool = ctx.enter_context(tc.tile_pool(name="dpool", bufs=1, space="PSUM"))

    # ---- resident weights (split DMAs: one writer per consumed slice) ----
    wt_sb = singles.tile([128, 8, 3 * HPC * DK], bf16)   # [p, dchunk, 1536]
    wt_r = wt.rearrange("(c p) f -> p c f", p=128)
    for d in range(8):
        for wv in range(3):
            nc.sync.dma_start(
                out=wt_sb[:, d, wv * 512:(wv + 1) * 512],
                in_=wt_r[:, d, wv * 512:(wv + 1) * 512],
            )
    rhs_sb = singles.tile([128, HPC, NJ, 65], bf16)      # [klow, h, j, 65]
    rhs_r = rhs.rearrange("h j p c -> p h j c")
    for h in range(HPC):
        for j in range(NJ):
            nc.sync.dma_start(out=rhs_sb[:, h, j, :], in_=rhs_r[:, h, j, :])
    ident = singles.tile([128, 128], bf16)
    nc.sync.dma_start(out=ident, in_=idin)

    # persistent dW.T accumulators: 4 psum banks, 2 heads each ([0:64],[64:128]).
    dw_ps = [
        dpool.tile([128, DKEY], f32, tag=f"dw{i}", name=f"dw{i}") for i in range(4)
    ]
    for i in range(4):
        nc.vector.memset(dw_ps[i], 0.0)

    st_state = {}   # tile index -> dict of carried tiles

    def stage_prev(p):
        """Emit tile p's transposes/copies/retrieval/vector-tail.
        Called at the top of iteration p+1."""
        mk = p["mk"]
        # -- vga first on the vector queue: inputs ready at iteration start --
        yv0 = p["yv"]
        vga0 = work.tile([128, HPC, DK], bf16, tag="vga", bufs=2, name="vga0")
        nc.vector.tensor_tensor(
            vga0, p["vg"], yv0.broadcast_to([128, HPC, DK]), op=Alu.mult
        )
        p["vga"] = vga0
        # -- PE transposes into ring slots; copies to SBUF --
        mkT = work.tile([128, HPC, NJ, 128], bf16, tag="mkT", bufs=2, name="mkT")
        for j in range(NJ):
            ps = ring.tile([128, HPC, 128], bf16, tag="ring", name=f"psT{j}")
            for h in range(HPC):
                nc.tensor.transpose(ps[:, h, :], mk[:, h, j, :], ident)
            if j == 2:
                nc.vector.tensor_copy(mkT[:, :, j, :], ps)
            else:
                nc.scalar.copy(mkT[:, :, j, :], ps)

        # -- retrieval --
        psR0 = ring.tile([128, 4, 65], f32, tag="ring", name="psR0")
        psR1 = ring.tile([128, 4, 65], f32, tag="ring", name="psR1")
        for j in range(NJ):
            for h in range(HPC):
                pr = psR0 if h < 4 else psR1
                nc.tensor.matmul(
                    pr[:, h % 4, :],
                    mkT[:, h, j, :],
                    rhs_sb[:, h, j, :],
                    start=(j == 0),
                    stop=(j == NJ - 1),
                )
        p["psR0"], p["psR1"] = psR0, psR1

    def stage_prev_tail(p):
        """Vector tail for tile p: beta = 1/zmk; mvg = vg*a - (num*beta*a)*g.
        Emitted after tile i's phi products so the vector queue stays in
        readiness order."""
        yv = p["yv"]
        psR0, psR1 = p["psR0"], p["psR1"]
        vga = p["vga"]
        ba = tiny.tile([128, HPC], bf16, tag="ba")
        for i2, pr in enumerate((psR0, psR1)):
            zc = tiny.tile([128, 4], f32, tag="zc", name=f"zc{i2}")
            nc.vector.tensor_scalar(zc, pr[:, :, 64], 1e-9, None, op0=Alu.max)
            be = tiny.tile([128, 4], f32, tag="be", name=f"be{i2}")
            nc.vector.reciprocal(be, zc)
            nc.vector.tensor_tensor(
                ba[:, 4 * i2:4 * i2 + 4], be, yv[:, 4 * i2:4 * i2 + 4],
                op=Alu.mult,
            )
        p1 = work.tile([128, HPC, DK], bf16, tag="p1", bufs=2)
        for i2, pr in enumerate((psR0, psR1)):
            nc.vector.tensor_tensor(
                p1[:, 4 * i2:4 * i2 + 4, :], pr[:, :, 0:64],
                ba[:, 4 * i2:4 * i2 + 4].broadcast_to([128, 4, DK]), op=Alu.mult
            )
        p1g = work.tile([128, HPC, DK], bf16, tag="p1g", bufs=2)
        nc.vector.tensor_tensor(p1g, p1, p["g"], op=Alu.mult)
        mvg = work.tile([128, HPC, DK], bf16, tag="mvg", bufs=2)
        nc.vector.tensor_tensor(mvg, vga, p1g, op=Alu.subtract)
        p["mvg"] = mvg

    def emit_outer(p):
        mvg, mk = p["mvg"], p["mk"]
        for h in range(HPC):
            nc.tensor.matmul(
                dw_ps[h // 2][64 * (h % 2):64 * (h % 2) + 64, :],
                mvg[:, h, :],
                mk[:, h, :, :],
                start=False,
                stop=False,
                skip_group_check=True,
                tile_position=(0, 64 * (h % 2)),
            )

    def load_x(i):
        s0 = i * ST
        x_sb = xpool.tile([128, 8, ST], bf16, name="x_sb")
        xt_r = xt[:, s0:s0 + ST].rearrange("(c p) s -> p c s", p=128)
        for d in range(8):
            nc.sync.dma_start(out=x_sb[:, d, :], in_=xt_r[:, d, :])
        return x_sb

    x_tiles = {0: load_x(0)}

    for i in range(NST + 1):
        cur = {}
        if i < NST:
            x_sb = x_tiles.pop(i)
            # ---- prefetch next tile's X one iteration ahead ----
            if i + 1 < NST:
                x_tiles[i + 1] = load_x(i + 1)

            # ---- K-wave ----
            psK = kbt.tile([128, 512], f32, tag="kbt", name="psK")
            for d in range(8):
                nc.tensor.matmul(
                    psK, x_sb[:, d, :], wt_sb[:, d, 0:512],
                    start=(d == 0), stop=(d == 7),
                )

        if i < NST:
            # ---- relus -> xx2 first on the scalar queue (psK ready early) ----
            xx2 = work.tile([128, HPC, 256], bf16, tag="xx2", bufs=2)
            kin = psK.rearrange("p (h f) -> p h f", h=HPC)
            for neg, off in ((False, 0), (True, 64)):
                dst = bass.AP(
                    tensor=xx2.tensor,
                    offset=xx2.offset + off,
                    ap=[xx2.ap[0], [256, HPC], [128, 2], [1, 64]],
                )
                src = bass.AP(
                    tensor=kin.tensor,
                    offset=kin.offset,
                    ap=[kin.ap[0], [64, HPC], [0, 2], [1, 64]],
                )
                if neg:
                    nc.scalar.activation(dst, src, Act.Relu, scale=-1.0)
                else:
                    nc.scalar.activation(dst, src, Act.Relu)

        # ---- tile i-1: transposes, copies, retrieval ----
        if i >= 1:
            stage_prev(st_state[i - 1])

        if i < NST:
            # ---- phi products: mk_j = xx * roll_j(xx) (direct views) ----
            mk = work.tile([128, HPC, NJ, 128], bf16, tag="mk")
            xx_c = xx2[:, :, 128:256]
            nc.gpsimd.tensor_tensor(
                mk[:, :, 0, :], xx_c, xx2[:, :, 127:255], op=Alu.mult
            )  # j=1 (odd offset)
            nc.vector.tensor_tensor(
                mk[:, :, 1, :], xx_c, xx2[:, :, 126:254], op=Alu.mult
            )  # j=2 (aligned, 2x)
            nc.vector.tensor_tensor(
                mk[:, :, 2, :], xx_c, xx2[:, :, 125:253], op=Alu.mult
            )  # j=3 (odd offset, 1x mode)
            cur["mk"] = mk

        # ---- tile i-1 vector tail (after tile i's phi products) ----
        if i >= 1:
            stage_prev_tail(st_state[i - 1])

        if i < NST:
            # ---- B-wave (kbt slot: waits relus drain psK) ----
            psB = kbt.tile([128, 512], f32, tag="kbt", name="psB")
            for d in range(8):
                nc.tensor.matmul(
                    psB, x_sb[:, d, :], wt_sb[:, d, 1024:1536],
                    start=(d == 0), stop=(d == 7),
                )

            # ---- sigmoid gate ----
            g_sb = work.tile([128, HPC, DK], bf16, tag="g")
            nc.scalar.activation(
                g_sb, psB.rearrange("p (h f) -> p h f", h=HPC), Act.Sigmoid
            )
            cur["g"] = g_sb

            # ---- V-wave (ring slot, after the stage's 5 allocations) ----
            psV = ring.tile([128, 512], f32, tag="ring", name="psV")
            for d in range(8):
                nc.tensor.matmul(
                    psV, x_sb[:, d, :], wt_sb[:, d, 512:1024],
                    start=(d == 0), stop=(d == 7),
                )

            # ---- vg = psV * g: drains psV this iteration ----
            vg = work.tile([128, HPC, DK], bf16, tag="vg")
            nc.vector.tensor_tensor(
                vg, psV.rearrange("p (h f) -> p h f", h=HPC), g_sb, op=Alu.mult
            )
            cur["vg"] = vg

        # ---- outer product of tile i-1 (after V/vg in tensor order) ----
        if i >= 1:
            emit_outer(st_state[i - 1])
            del st_state[i - 1]

        if i < NST:
            # ---- ss = ||mk||^2, split: heads 0-3 vector stt, 4-7 scalar ----
            sq = work.tile([128, HPC, DKEY], bf16, tag="sq", bufs=2)
            ss = tiny.tile([128, HPC], f32, tag="ss", bufs=2)
            mkf = mk.rearrange("p h j k -> p h (j k)")
            for h in range(4):
                nc.vector.scalar_tensor_tensor(
                    sq[:, h, :], mkf[:, h, :], 1.0, mkf[:, h, :],
                    op0=Alu.mult, op1=Alu.mult,
                    accum_out=ss[:, h:h + 1],
                )
            for h in range(4, HPC):
                nc.scalar.activation(
                    sq[:, h, :], mkf[:, h, :], Act.Square,
                    accum_out=ss[:, h:h + 1],
                )

            # ---- alpha = rsqrt(ss): fast inverse sqrt + 1 Newton ----
            t0 = tiny.tile([128, HPC], f32, tag="t0", bufs=2)
            nc.vector.tensor_scalar(t0, ss, 1e-20, None, op0=Alu.max)
            yv = tiny.tile([128, HPC], f32, tag="yv", bufs=3)
            sh = tiny.tile([128, HPC], f32, tag="sh", bufs=2)
            nc.vector.tensor_scalar(
                sh.bitcast(i32), t0.bitcast(i32), 1, None,
                op0=Alu.logical_shift_right
            )
            nc.vector.tensor_scalar(
                yv.bitcast(i32), sh.bitcast(i32), -1, 0x5F3759DF,
                op0=Alu.mult, op1=Alu.add,
            )
            aa = tiny.tile([128, HPC], f32, tag="aa", bufs=2)
            bb = tiny.tile([128, HPC], f32, tag="bb", bufs=2)
            nc.vector.tensor_tensor(aa, yv, yv, op=Alu.mult)
            nc.vector.tensor_tensor(bb, aa, t0, op=Alu.mult)
            nc.vector.tensor_scalar(bb, bb, -0.5, 1.5, op0=Alu.mult, op1=Alu.add)
            nc.vector.tensor_tensor(yv, yv, bb, op=Alu.mult)
            cur["yv"] = yv

            st_state[i] = cur

    # ---- write out dW.T (PSUM -> SBUF -> DRAM) ----
    for i in range(4):
        dwsb = work.tile([128, DKEY], f32, tag="dwsb", bufs=1, name=f"dwsb{i}")
        nc.vector.tensor_copy(dwsb, dw_ps[i])
        nc.sync.dma_start(
            out=out_dwt[2 * i:2 * i + 2].rearrange("h v k -> (h v) k"),
            in_=dwsb,
        )


def _build():
    import concourse.bass as bass
    import concourse.tile as tile
    from concourse import mybir

    nc = bass.Bass(trn_type="TRN2", num_devices=NCORES)
    xt = nc.dram_tensor("xt", (D, S), mybir.dt.bfloat16, kind="ExternalInput").ap()
    wt = nc.dram_tensor(
        "wt", (D, 3 * HPC * DK), mybir.dt.bfloat16, kind="ExternalInput"
    ).ap()
    rhs = nc.dram_tensor(
        "rhs", (HPC, NJ, 128, 65), mybir.dt.bfloat16, kind="ExternalInput"
    ).ap()
    idin = nc.dram_tensor(
        "ident", (128, 128), mybir.dt.bfloat16, kind="ExternalInput"
    ).ap()
    out = nc.dram_tensor(
        "dwt", (HPC, DV, DKEY), mybir.dt.float32, kind="ExternalOutput"
    ).ap()
    with tile.TileContext(nc) as tc:
        with ExitStack() as ctx:
            _body(ctx, tc, out, xt, wt, rhs, idin)
    n = _split_excess_waits(nc)
    print(f"[kernel] split {n} excess-wait chunks onto drains")
    return nc


_CACHE = {}


def _prep_core_inputs(mem_tokens, W_mk, W_mv, W_mb, W_mem, z):
    """Build the 8 per-core input maps (host-side shard + layout prep)."""
    bf = ml_dtypes.bfloat16
    ident = np.eye(128, dtype=np.float32).astype(bf)
    in_maps = []
    for c in range(NCORES):
        b = c // 2
        h0 = (c % 2) * HPC
        xt = np.ascontiguousarray(mem_tokens[b].T).astype(bf)        # [D, S]
        ws = []
        for W in (W_mk, W_mv, W_mb):
            ws.append(W[h0 * DK:(h0 + HPC) * DK, :])                 # [512, D]
        wt = np.ascontiguousarray(np.concatenate(ws, axis=0).T).astype(bf)
        rhs = np.zeros((HPC, NJ, 128, 65), dtype=np.float32)
        wm = W_mem[b, h0:h0 + HPC]                                   # [8, 384, 64]
        zz = z[b, h0:h0 + HPC]                                       # [8, 384]
        for j in range(NJ):
            rhs[:, j, :, 0:64] = wm[:, j * 128:(j + 1) * 128, :]
            rhs[:, j, :, 64] = zz[:, j * 128:(j + 1) * 128]
        in_maps.append(
            {"xt": xt, "wt": wt, "rhs": rhs.astype(bf), "ident": ident}
        )
    return in_maps


def kernel(mem_tokens, W_mk, W_mv, W_mb, W_mem, z, _want_profile=False):
    from concourse.bass_utils import run_bass_kernel_spmd

    if "nc" not in _CACHE:
        _CACHE["nc"] = _build()
    nc = _CACHE["nc"]
    in_maps = _prep_core_inputs(mem_tokens, W_mk, W_mv, W_mb, W_mem, z)
    res = run_bass_kernel_spmd(
        nc, in_maps, core_ids=list(range(NCORES)), trace=_want_profile
    )
    out = np.empty((B, 16, DKEY, DV), dtype=np.float32)
    for c in range(NCORES):
        b = c // 2
        h0 = (c % 2) * HPC
        dwt = res.results[c]["dwt"]                                  # [8, 64, 384]
        out[b, h0:h0 + HPC] = np.transpose(dwt, (0, 2, 1))
    out += W_mem.astype(np.float32)
    if _want_profile:
        return out, res
    return out


# revision 41
# speedup vs baseline: 1.1824x; 1.1824x over previous
"""Trainium2 Bass kernel for the scatter_memory delta-rule module (v3).

Computation (per batch b, head h):
  Y = X @ [W_mk|W_mv|W_mb].T            (X = mem_tokens[b], [S, D])
  k_raw, new_mv, mb_raw = per-head 64-col slices of Y
  xx  = [relu(k), relu(-k)]             ([S, 128])
  mk_j = xx * roll_j(xx), j=1..3        (mk = [S, 384], all >= 0)
  ss  = ||mk||^2, alpha = rsqrt(ss)
  num = mk @ W_mem, zmk = mk @ z        (retrieval)
  prev = num / zmk   (the reference's 1e-5 eps term is <= 1e-5 relative
                      since zmk = L1(mk) >= L2(mk) = r; dropped)
  mvg = (new_mv - prev) * sigmoid(mb_raw) * alpha
  dW  = mk.T @ mvg ;  out = W_mem + dW

v3: software-pipelined by one tile. Iteration i runs tile i's
projections/relus/phi on tensor/scalar/gpsimd while tile i-1's PE
transposes, PSUM->SBUF copies, retrieval and vector tail execute, and
tile i-2's state has fully drained. PSUM: 4 dW accumulator banks, 1
K/B bank, 3 ring banks cycling [T0 T1 T2 R0 R1 V] so every slot-reuse
dependency is at least one iteration stale. psV is drained early by
vg = psV*g (sigmoid gate available same iteration); alpha lands on the
v-side one iteration later as mvg = vg*alpha - (num*beta*alpha)*g.

Sharding: 8 cores = (4 batches) x (2 half-head groups of 8 heads).
Host prep: X transposed to [D, S] bf16; weight slices pre-transposed;
W_mem/z packed to a [H, 3, 128, 65] rhs. Device returns dW.T [H, 64, 384]
fp32; host transposes and adds W_mem in fp32.
"""

import numpy as np
import ml_dtypes
from contextlib import ExitStack

def _split_excess_waits(nc, max_waits=1, drain_waits=1):
    """The walrus build here encodes only ONE sync wait per instruction
    (updates are separate). Move excess waits onto prepended same-engine
    drains, one wait each."""
    from concourse import mybir

    ctr = [0]
    for f in nc.m.functions:
        for bb in f.blocks:
            il = list(bb.instructions)
            out = []
            changed = False
            for inst in il:
                si = getattr(inst, "sync_info", None)
                waits = list(si.on_wait) if si and si.on_wait else []
                ups = list(si.on_update) if si and si.on_update else []
                if len(waits) > max_waits:
                    keep = waits[:max_waits]
                    rest = waits[max_waits:]
                    for i in range(0, len(rest), drain_waits):
                        chunk = rest[i:i + drain_waits]
                        ctr[0] += 1
                        d = mybir.InstDrain(
                            name=f"waitsplit{ctr[0]}",
                            ins=[],
                            outs=[],
                            bass_is_fusable=False,
                        )
                        d.engine = inst.engine
                        d.sync_info = mybir.SyncInfo(on_wait=chunk, on_update=[])
                        out.append(d)
                    inst.sync_info = mybir.SyncInfo(on_wait=keep, on_update=ups)
                    changed = True
                out.append(inst)
            if changed:
                bb.instructions = out
    return ctr[0]

B, S, D = 4, 4096, 1024
HPC = 8            # heads per core
NCORES = 8
DK = 64            # dk per head
DKEY = 384         # 2*nu*dk
DV = 64
ST = 128           # tokens per tile
NST = S // ST      # 32
NJ = 3


def _body(ctx, tc, out_dwt, xt, wt, rhs, idin):
    import concourse.bass as bass
    from concourse import mybir

    nc = tc.nc
    bf16 = mybir.dt.bfloat16
    f32 = mybir.dt.float32
    i32 = mybir.dt.int32
    Alu = mybir.AluOpType
    Act = mybir.ActivationFunctionType

    singles = ctx.enter_context(tc.tile_pool(name="singles", bufs=1))
    xpool = ctx.enter_context(tc.tile_pool(name="xpool", bufs=3))
    work = ctx.enter_context(tc.tile_pool(name="work", bufs=3))
    tiny = ctx.enter_context(tc.tile_pool(name="tiny", bufs=4))
    kbt = ctx.enter_context(tc.tile_pool(name="kbt", bufs=1, space="PSUM"))
    ring = ctx.enter_context(tc.tile_pool(name="ring", bufs=3, space="PSUM"))
    dpool = ctx.enter_context(tc.tile_pool(name="dpool", bufs=1, space="PSUM"))

    # ---- resident weights (split DMAs: one writer per consumed slice) ----
    wt_sb = singles.tile([128, 8, 3 * HPC * DK], bf16)   # [p, dchunk, 1536]
    wt_r = wt.rearrange("(c p) f -> p c f", p=128)
    for d in range(8):
        for wv in range(3):
            nc.sync.dma_start(
                out=wt_sb[:, d, wv * 512:(wv + 1) * 512],
                in_=wt_r[:, d, wv * 512:(wv + 1) * 512],
            )
    rhs_sb = singles.tile([128, HPC, NJ, 65], bf16)      # [klow, h, j, 65]
    rhs_r = rhs.rearrange("h j p c -> p h j c")
    for h in range(HPC):
        for j in range(NJ):
            nc.sync.dma_start(out=rhs_sb[:, h, j, :], in_=rhs_r[:, h, j, :])
    ident = singles.tile([128, 128], bf16)
    nc.sync.dma_start(out=ident, in_=idin)

    # persistent dW.T accumulators: 4 psum banks, 2 heads each ([0:64],[64:128]).
    dw_ps = [
        dpool.tile([128, DKEY], f32, tag=f"dw{i}", name=f"dw{i}") for i in range(4)
    ]
    for i in range(4):
        nc.vector.memset(dw_ps[i], 0.0)

    st_state = {}   # tile index -> dict of carried tiles

    def stage_prev(p):
        """Emit tile p's transposes/copies/retrieval/vector-tail.
        Called at the top of iteration p+1."""
        mk = p["mk"]
        # -- vga first on the vector queue: inputs ready at iteration start --
        yv0 = p["yv"]
        vga0 = work.tile([128, HPC, DK], bf16, tag="vga", bufs=2, name="vga0")
        nc.vector.tensor_tensor(
            vga0, p["vg"], yv0.broadcast_to([128, HPC, DK]), op=Alu.mult
        )
        p["vga"] = vga0
        # -- PE transposes into ring slots; copies to SBUF --
        mkT = work.tile([128, HPC, NJ, 128], bf16, tag="mkT", bufs=2, name="mkT")
        for j in range(NJ):
            ps = ring.tile([128, HPC, 128], bf16, tag="ring", name=f"psT{j}")
            for h in range(HPC):
                nc.tensor.transpose(ps[:, h, :], mk[:, h, j, :], ident)
            if j == 2:
                nc.vector.tensor_copy(mkT[:, :, j, :], ps)
            else:
                nc.scalar.copy(mkT[:, :, j, :], ps)

        # -- retrieval --
        psR0 = ring.tile([128, 4, 65], f32, tag="ring", name="psR0")
        psR1 = ring.tile([128, 4, 65], f32, tag="ring", name="psR1")
        for j in range(NJ):
            for h in range(HPC):
                pr = psR0 if h < 4 else psR1
                nc.tensor.matmul(
                    pr[:, h % 4, :],
                    mkT[:, h, j, :],
                    rhs_sb[:, h, j, :],
                    start=(j == 0),
                    stop=(j == NJ - 1),
                )
        p["psR0"], p["psR1"] = psR0, psR1

    def stage_prev_tail(p):
        """Vector tail for tile p: beta = 1/zmk; mvg = vg*a - (num*beta*a)*g.
        Emitted after tile i's phi products so the vector queue stays in
        readiness order."""
        yv = p["yv"]
        psR0, psR1 = p["psR0"], p["psR1"]
        vga = p["vga"]
        ba = tiny.tile([128, HPC], bf16, tag="ba")
        for i2, pr in enumerate((psR0, psR1)):
            zc = tiny.tile([128, 4], f32, tag="zc", name=f"zc{i2}")
            nc.vector.tensor_scalar(zc, pr[:, :, 64], 1e-9, None, op0=Alu.max)
            be = tiny.tile([128, 4], f32, tag="be", name=f"be{i2}")
            nc.vector.reciprocal(be, zc)
            nc.vector.tensor_tensor(
                ba[:, 4 * i2:4 * i2 + 4], be, yv[:, 4 * i2:4 * i2 + 4],
                op=Alu.mult,
            )
        p1 = work.tile([128, HPC, DK], bf16, tag="p1", bufs=2)
        for i2, pr in enumerate((psR0, psR1)):
            nc.vector.tensor_tensor(
                p1[:, 4 * i2:4 * i2 + 4, :], pr[:, :, 0:64],
                ba[:, 4 * i2:4 * i2 + 4].broadcast_to([128, 4, DK]), op=Alu.mult
            )
        p1g = work.tile([128, HPC, DK], bf16, tag="p1g", bufs=2)
        nc.vector.tensor_tensor(p1g, p1, p["g"], op=Alu.mult)
        mvg = work.tile([128, HPC, DK], bf16, tag="mvg", bufs=2)
        nc.vector.tensor_tensor(mvg, vga, p1g, op=Alu.subtract)
        p["mvg"] = mvg

    def emit_outer(p):
        mvg, mk = p["mvg"], p["mk"]
        for h in range(HPC):
            nc.tensor.matmul(
                dw_ps[h // 2][64 * (h % 2):64 * (h % 2) + 64, :],
                mvg[:, h, :],
                mk[:, h, :, :],
                start=False,
                stop=False,
                skip_group_check=True,
                tile_position=(0, 64 * (h % 2)),
            )

    def load_x(i):
        s0 = i * ST
        x_sb = xpool.tile([128, 8, ST], bf16, name="x_sb")
        xt_r = xt[:, s0:s0 + ST].rearrange("(c p) s -> p c s", p=128)
        for d in range(8):
            nc.sync.dma_start(out=x_sb[:, d, :], in_=xt_r[:, d, :])
        return x_sb

    x_tiles = {0: load_x(0)}

    for i in range(NST + 1):
        cur = {}
        if i < NST:
            x_sb = x_tiles.pop(i)
            # ---- prefetch next tile's X one iteration ahead ----
            if i + 1 < NST:
                x_tiles[i + 1] = load_x(i + 1)

            # ---- K-wave ----
            psK = kbt.tile([128, 512], f32, tag="kbt", name="psK")
            for d in range(8):
                nc.tensor.matmul(
                    psK, x_sb[:, d, :], wt_sb[:, d, 0:512],
                    start=(d == 0), stop=(d == 7),
                )

        if i < NST:
            # ---- relus -> xx2 first on the scalar queue (psK ready early) ----
            xx2 = work.tile([128, HPC, 256], bf16, tag="xx2", bufs=2)
            kin = psK.rearrange("p (h f) -> p h f", h=HPC)
            for neg, off in ((False, 0), (True, 64)):
                dst = bass.AP(
                    tensor=xx2.tensor,
                    offset=xx2.offset + off,
                    ap=[xx2.ap[0], [256, HPC], [128, 2], [1, 64]],
                )
                src = bass.AP(
                    tensor=kin.tensor,
                    offset=kin.offset,
                    ap=[kin.ap[0], [64, HPC], [0, 2], [1, 64]],
                )
                if neg:
                    nc.scalar.activation(dst, src, Act.Relu, scale=-1.0)
                else:
                    nc.scalar.activation(dst, src, Act.Relu)

        # ---- tile i-1: transposes, copies, retrieval ----
        if i >= 1:
            stage_prev(st_state[i - 1])

        if i < NST:
            # ---- phi products: mk_j = xx * roll_j(xx) (direct views) ----
            mk = work.tile([128, HPC, NJ, 128], bf16, tag="mk")
            xx_c = xx2[:, :, 128:256]
            nc.gpsimd.tensor_tensor(
                mk[:, :, 0, :], xx_c, xx2[:, :, 127:255], op=Alu.mult
            )  # j=1 (odd offset)
            nc.vector.tensor_tensor(
                mk[:, :, 1, :], xx_c, xx2[:, :, 126:254], op=Alu.mult
            )  # j=2 (aligned, 2x)
            nc.vector.tensor_tensor(
                mk[:, :, 2, :], xx_c, xx2[:, :, 125:253], op=Alu.mult
            )  # j=3 (odd offset, 1x mode)
            cur["mk"] = mk

        # ---- tile i-1 vector tail (after tile i's phi products) ----
        if i >= 1:
            stage_prev_tail(st_state[i - 1])

        if i < NST:
            # ---- B-wave (kbt slot: waits relus drain psK) ----
            psB = kbt.tile([128, 512], f32, tag="kbt", name="psB")
            for d in range(8):
                nc.tensor.matmul(
                    psB, x_sb[:, d, :], wt_sb[:, d, 1024:1536],
                    start=(d == 0), stop=(d == 7),
                )

            # ---- sigmoid gate ----
            g_sb = work.tile([128, HPC, DK], bf16, tag="g")
            nc.scalar.activation(
                g_sb, psB.rearrange("p (h f) -> p h f", h=HPC), Act.Sigmoid
            )
            cur["g"] = g_sb

            # ---- V-wave (ring slot, after the stage's 5 allocations) ----
            psV = ring.tile([128, 512], f32, tag="ring", name="psV")
            for d in range(8):
                nc.tensor.matmul(
                    psV, x_sb[:, d, :], wt_sb[:, d, 512:1024],
                    start=(d == 0), stop=(d == 7),
                )

            # ---- vg = psV * g: drains psV this iteration ----
            vg = work.tile([128, HPC, DK], bf16, tag="vg")
            nc.vector.tensor_tensor(
                vg, psV.rearrange("p (h f) -> p h f", h=HPC), g_sb, op=Alu.mult
            )
            cur["vg"] = vg

        # ---- outer product of tile i-1 (after V/vg in tensor order) ----
        if i >= 1:
            emit_outer(st_state[i - 1])
            del st_state[i - 1]

        if i < NST:
            # ---- ss = ||mk||^2, split: heads 0-3 vector stt, 4-7 scalar ----
            sq = work.tile([128, HPC, DKEY], bf16, tag="sq", bufs=2)
            ss = tiny.tile([128, HPC], f32, tag="ss", bufs=2)
            mkf = mk.rearrange("p h j k -> p h (j k)")
            for h in range(4):
                nc.vector.scalar_tensor_tensor(
                    sq[:, h, :], mkf[:, h, :], 1.0, mkf[:, h, :],
                    op0=Alu.mult, op1=Alu.mult,
                    accum_out=ss[:, h:h + 1],
                )
            for h in range(4, HPC):
                nc.scalar.activation(
                    sq[:, h, :], mkf[:, h, :], Act.Square,
                    accum_out=ss[:, h:h + 1],
                )

            # ---- alpha = rsqrt(ss): fast inverse sqrt + 1 Newton ----
            t0 = tiny.tile([128, HPC], f32, tag="t0", bufs=2)
            nc.vector.tensor_scalar(t0, ss, 1e-20, None, op0=Alu.max)
            yv = tiny.tile([128, HPC], f32, tag="yv", bufs=3)
            sh = tiny.tile([128, HPC], f32, tag="sh", bufs=2)
            nc.vector.tensor_scalar(
                sh.bitcast(i32), t0.bitcast(i32), 1, None,
                op0=Alu.logical_shift_right
            )
            nc.vector.tensor_scalar(
                yv.bitcast(i32), sh.bitcast(i32), -1, 0x5F3759DF,
                op0=Alu.mult, op1=Alu.add,
            )
            aa = tiny.tile([128, HPC], f32, tag="aa", bufs=2)
            bb = tiny.tile([128, HPC], f32, tag="bb", bufs=2)
            nc.vector.tensor_tensor(aa, yv, yv, op=Alu.mult)
            nc.vector.tensor_tensor(bb, aa, t0, op=Alu.mult)
            nc.vector.tensor_scalar(bb, bb, -0.5, 1.5, op0=Alu.mult, op1=Alu.add)
            nc.vector.tensor_tensor(yv, yv, bb, op=Alu.mult)
            cur["yv"] = yv

            st_state[i] = cur

    # ---- write out dW.T (PSUM -> SBUF -> DRAM) ----
    for i in range(4):
        dwsb = work.tile([128, DKEY], f32, tag="dwsb", bufs=1, name=f"dwsb{i}")
        nc.vector.tensor_copy(dwsb, dw_ps[i])
        nc.sync.dma_start(
            out=out_dwt[2 * i:2 * i + 2].rearrange("h v k -> (h v) k"),
            in_=dwsb,
        )


def _build():
    import concourse.bass as bass
    import concourse.tile as tile
    from concourse import mybir

    nc = bass.Bass(trn_type="TRN2", num_devices=NCORES)
    xt = nc.dram_tensor("xt", (D, S), mybir.dt.bfloat16, kind="ExternalInput").ap()
    wt = nc.dram_tensor(
        "wt", (D, 3 * HPC * DK), mybir.dt.bfloat16, kind="ExternalInput"
    ).ap()
    rhs = nc.dram_tensor(
        "rhs", (HPC, NJ, 128, 65), mybir.dt.bfloat16, kind="ExternalInput"
    ).ap()
    idin = nc.dram_tensor(
        "ident", (128, 128), mybir.dt.bfloat16, kind="ExternalInput"
    ).ap()
    out = nc.dram_tensor(
        "dwt", (HPC, DV, DKEY), mybir.dt.float32, kind="ExternalOutput"
    ).ap()
    with tile.TileContext(nc) as tc:
        with ExitStack() as ctx:
            _body(ctx, tc, out, xt, wt, rhs, idin)
    n = _split_excess_waits(nc)
    print(f"[kernel] split {n} excess-wait chunks onto drains")
    return nc


_CACHE = {}


def _prep_core_inputs(mem_tokens, W_mk, W_mv, W_mb, W_mem, z):
    """Build the 8 per-core input maps (host-side shard + layout prep)."""
    bf = ml_dtypes.bfloat16
    ident = np.eye(128, dtype=np.float32).astype(bf)
    in_maps = []
    for c in range(NCORES):
        b = c // 2
        h0 = (c % 2) * HPC
        xt = np.ascontiguousarray(mem_tokens[b].T).astype(bf)        # [D, S]
        ws = []
        for W in (W_mk, W_mv, W_mb):
            ws.append(W[h0 * DK:(h0 + HPC) * DK, :])                 # [512, D]
        wt = np.ascontiguousarray(np.concatenate(ws, axis=0).T).astype(bf)
        rhs = np.zeros((HPC, NJ, 128, 65), dtype=np.float32)
        wm = W_mem[b, h0:h0 + HPC]                                   # [8, 384, 64]
        zz = z[b, h0:h0 + HPC]                                       # [8, 384]
        for j in range(NJ):
            rhs[:, j, :, 0:64] = wm[:, j * 128:(j + 1) * 128, :]
            rhs[:, j, :, 64] = zz[:, j * 128:(j + 1) * 128]
        in_maps.append(
            {"xt": xt, "wt": wt, "rhs": rhs.astype(bf), "ident": ident}
        )
    return in_maps


def kernel(mem_tokens, W_mk, W_mv, W_mb, W_mem, z, _want_profile=False):
    from concourse.bass_utils import run_bass_kernel_spmd

    if "nc" not in _CACHE:
        _CACHE["nc"] = _build()
    nc = _CACHE["nc"]
    in_maps = _prep_core_inputs(mem_tokens, W_mk, W_mv, W_mb, W_mem, z)
    res = run_bass_kernel_spmd(
        nc, in_maps, core_ids=list(range(NCORES)), trace=_want_profile
    )
    out = np.empty((B, 16, DKEY, DV), dtype=np.float32)
    for c in range(NCORES):
        b = c // 2
        h0 = (c % 2) * HPC
        dwt = res.results[c]["dwt"]                                  # [8, 64, 384]
        out[b, h0:h0 + HPC] = np.transpose(dwt, (0, 2, 1))
    out += W_mem.astype(np.float32)
    if _want_profile:
        return out, res
    return out
